# revision 18
# baseline (speedup 1.0000x reference)
"""Trainium2 Bass kernel for nn_Conv2d_14147622273082.

Conv2d 3x3, stride 1, pad 1: x [8, 320, 64, 64] f32, hf8-coded weights
w_bits [320, 320, 3, 3] i32 (codes 0..255), bias codes b_bits [320] i32.
out = conv2d(x, hf8_decode(w_bits)) + hf8_decode(b_bits).

Strategy: data-parallel over batch (1 image per NeuronCore, 8 cores).
hf8 decode is a 256-entry LUT done host-side; weights are replicated.

Mixed fp8/fp16 matmul stream. The PE's DoubleRow fp8 mode contracts 256
rows per MM at the same issue rate as a 128-row fp16 MM (measured 222ns
either way at N=512), i.e. 2x throughput. Pure-fp8 x quantization costs
2.65% relative error (3 mantissa bits) vs the 2e-2 gate, so only P8=5 of
the 9 kernel positions (for channels 0:256) run in fp8 DoubleRow
(measured 1.77e-2 end-to-end); the rest stay fp16. Scales: weights are
hf8*64 in e4m3 (max exactly 240 = TRN e4m3 max normal, exact), x*32 in
e4m3, fp16 weights *2048 (power-of-2, exact) so both paths accumulate at
2048x in PSUM; the epilogue activation applies scale=1/2048 + bias and
writes fp16 (output downcast is ~2.4e-4, negligible vs 1.77e-2).

Per [128cout, 512pix] tile: 5 DR MMs (x8 stacked [128,2,66,72], blocks =
ch 0:128 / 128:256) + 8 fp16 MMs (pos 5..8 on xp0/xp1) + 4 tail-pair MMs
+ 1/2 row-tiled solo = 17.5 slots vs 22.5 all-fp16. Cout tail 256:320
is col-tiled fp16 pixel-tile pairs as before (DoubleRow and column
tiling are mutually exclusive on the XBUS budget).
"""

import numpy as np
import ml_dtypes

import concourse.bass as bass
import concourse.tile as tile
from concourse import bacc, mybir
from concourse.bass_utils import run_bass_kernel_spmd

B, CIN, COUT, H, W = 8, 320, 320, 64, 64
PIX = H * W  # 4096
P = 128
HP, WP = H + 2, W + 4  # 66 x 68 fp16 padded image
WP8 = 72  # fp8 stacked image width: 66*72 bytes per plane, %16 == 0
NT = 512  # pixels per psum tile = 8 rows of 64
RPT = NT // W  # 8
NPT = PIX // NT  # 8
P8 = 7  # kernel positions 0..P8-1 of ch 0:256 go fp8 DoubleRow
# tail position pairing: pos = kh*3+kw; pairs (a, b) packed on partitions
# (0:64, 64:128). Pairs with flat-offset delta +1 use xp2 (lower half
# pre-shifted +1 col); the (2,3) pair has delta +66 and uses xb2.
TAIL_PAIRS = [(0, 1), (2, 3), (4, 5), (6, 7)]
N_ACC = P8 + 2 * (9 - P8) + len(TAIL_PAIRS) + 1  # 18
N_ACC2 = 2 * 9 + len(TAIL_PAIRS) + 1  # 23 (col-tiled cout-tail chunk)

F16 = mybir.dt.float16
F32 = mybir.dt.float32
F8 = mybir.dt.float8e4
DR = mybir.MatmulPerfMode.DoubleRow
N_WARM = 44  # covers preamble->first-DMA latency (~10.5us data gate) so the
# PE stays busy from ~7.5us and HAM un-throttles early in the real stream
WSCALE = 2048.0  # common PSUM scale: fp8 path 64*32, fp16 weights *2048


def _hf8_lut():
    bits = np.arange(256, dtype=np.int64)
    sign = np.where(((bits >> 7) & 1) == 1, -1.0, 1.0)
    exp = (bits >> 3) & 0xF
    man = (bits & 0x7).astype(np.float64)
    val = sign * np.where(
        exp == 0, 2.0 ** (1 - 14) * (man / 8.0), np.exp2(exp - 14.0) * (1 + man / 8.0)
    )
    return val


_LUT8 = (_hf8_lut() * 64.0).astype(ml_dtypes.float8_e4m3)  # max exactly 240
_LUT16 = (_hf8_lut() * WSCALE).astype(np.float16)  # exact (pow2 scale)
_LUT32 = _hf8_lut().astype(np.float32)


def build():
    from concourse.tile_rust import add_dep_helper

    nc = bacc.Bacc(
        "TRN2", target_bir_lowering=False, debug=False, enable_partition_id=False
    )
    x8_d = nc.dram_tensor("x8", [P, 2, HP, WP8], F8, kind="ExternalInput")
    xp_d = [
        nc.dram_tensor(f"xp{i}", [P, HP, WP], F16, kind="ExternalInput")
        for i in range(3)
    ]
    xb_d = nc.dram_tensor("xb2", [P, HP, WP], F16, kind="ExternalInput")
    xc_d = nc.dram_tensor("xc2", [64, HP, WP], F16, kind="ExternalInput")
    w8_d = nc.dram_tensor("w8", [P, P8, 2, COUT], F8, kind="ExternalInput")
    w0_d = nc.dram_tensor("w0", [P, 9 - P8, COUT], F16, kind="ExternalInput")
    w1_d = nc.dram_tensor("w1", [P, 9 - P8, COUT], F16, kind="ExternalInput")
    w0t_d = nc.dram_tensor("w0t", [P, P8, 64], F16, kind="ExternalInput")
    w1t_d = nc.dram_tensor("w1t", [P, P8, 64], F16, kind="ExternalInput")
    w2_d = nc.dram_tensor("w2", [P, 5, COUT], F16, kind="ExternalInput")
    bf_d = nc.dram_tensor("bf", [P, 4], F32, kind="ExternalInput")
    out_d = nc.dram_tensor("out", [COUT, PIX], F16, kind="ExternalOutput")

    with tile.TileContext(nc) as tc:
        with (
            tc.tile_pool(name="persist", bufs=1) as persist,
            tc.tile_pool(name="stage", bufs=1) as stage,
            tc.tile_pool(name="outsb", bufs=4) as outsb,
            tc.tile_pool(name="psum", bufs=1, space="PSUM") as pp,
        ):
            x8t = persist.tile([P, 2, HP, WP8], F8, tag="x8t", name="x8t")
            xt = [
                persist.tile([P, HP, WP], F16, tag=f"xt{i}", name=f"xt{i}")
                for i in range(5)
            ]
            w8 = persist.tile([P, P8, 2, COUT], F8, tag="w8", name="w8")
            wl0 = persist.tile([P, 9 - P8, COUT], F16, tag="wl0", name="wl0")
            wl1 = persist.tile([P, 9 - P8, COUT], F16, tag="wl1", name="wl1")
            w0t = persist.tile([P, P8, 64], F16, tag="w0t", name="w0t")
            w1t = persist.tile([P, P8, 64], F16, tag="w1t", name="w1t")
            wpair = persist.tile([P, 5, COUT], F16, tag="wpair", name="wpair")
            bf = persist.tile([P, 4], F32, tag="bf", name="bf")

            # ---- engine warmups (no data deps) ----
            wsrc = stage.tile([P, P], F16, tag="wsrc", name="wsrc")
            zsrc = stage.tile([P, 1], F32, tag="zsrc", name="zsrc")
            zo = stage.tile([P, 1], F32, tag="zo", name="zo")
            m0 = nc.gpsimd.memset(wsrc[:], 0.0)
            m1 = nc.gpsimd.memset(zsrc[:], 0.0)
            add_dep_helper(m1.ins, m0.ins, sync=False, reason="gpsimd order")
            act_warm = nc.scalar.activation(
                zo[:], zsrc[:], mybir.ActivationFunctionType.Identity, scale=1.0
            )

            # ---- input DMAs, deadline order. The ramp-critical transfers
            # go on the scalar queue: its descriptor processing (~650ns
            # each) runs in parallel with the sync queue's, and they all
            # complete before the MM stream starts (no SBUF contention) ----
            nc.scalar.dma_start(x8t[:, :, 0:8], x8_d[:, :, 0:8])
            nc.scalar.dma_start(w8[:, 0:1], w8_d[:, 0:1])
            nc.scalar.dma_start(x8t[:, :, 8:22], x8_d[:, :, 8:22])
            nc.scalar.dma_start(w8[:, 1:3], w8_d[:, 1:3])
            nc.scalar.dma_start(x8t[:, :, 22:34], x8_d[:, :, 22:34])
            nc.sync.dma_start(w8[:, 3:5], w8_d[:, 3:5])
            nc.sync.dma_start(w8[:, 5:P8], w8_d[:, 5:P8])
            nc.sync.dma_start(x8t[:, :, 34:50], x8_d[:, :, 34:50])
            nc.sync.dma_start(x8t[:, :, 50:66], x8_d[:, :, 50:66])
            nc.sync.dma_start(xt[0][:, 0:16], xp_d[0][:, 0:16])
            nc.sync.dma_start(wl0[:], w0_d[:])
            nc.sync.dma_start(xt[0][:, 16:40], xp_d[0][:, 16:40])
            nc.sync.dma_start(xt[0][:, 40:66], xp_d[0][:, 40:66])
            nc.sync.dma_start(wl1[:], w1_d[:])
            nc.sync.dma_start(xt[1][:], xp_d[1][:])
            nc.sync.dma_start(wpair[:], w2_d[:])
            nc.sync.dma_start(xt[2][:], xp_d[2][:])
            nc.sync.dma_start(xt[3][:], xb_d[:])
            nc.sync.dma_start(xt[4][64:P], xc_d[:])
            nc.sync.dma_start(w0t[:], w0t_d[:])
            nc.sync.dma_start(w1t[:], w1t_d[:])
            nc.sync.dma_start(bf[:], bf_d[:])
            warm_ps = pp.tile([P, NT], F32, tag="acc7", name="warm_ps")
            for _ in range(N_WARM):
                nc.tensor.matmul(
                    warm_ps[0:64, 0:64], wsrc[:, 0:64], wsrc[:, 0:64],
                    start=True, stop=True,
                )

            # ---- matmul stream ----
            prev_act = {"a": act_warm}

            def epi(acc_t, bias_col, dsts, queues=None):
                osb = outsb.tile([P, NT], F16, tag="osb", name="osb")
                a = nc.scalar.activation(
                    osb[:], acc_t,
                    mybir.ActivationFunctionType.Identity,
                    bias=bf[:, bias_col : bias_col + 1], scale=1.0 / WSCALE,
                )
                add_dep_helper(
                    a.ins, prev_act["a"].ins, sync=False, reason="epi order"
                )
                prev_act["a"] = a
                for qi, (dst, rows) in enumerate(dsts):
                    q = queues[qi] if queues else nc.sync
                    q.dma_start(dst, osb[rows[0] : rows[1]])

            def full_chunk(ms, mi, staged):
                acc = [
                    pp.tile([P, NT], F32, tag=f"acc{t}", name=f"acc_{mi}_{t}")
                    for t in range(NPT)
                ]
                cnt = [0] * NPT

                def mm8(pos, t):
                    # DoubleRow fp8: contracts ch 0:256 for one position
                    kh, kw = pos // 3, pos % 3
                    h0 = t * RPT
                    rhs = x8t[:, :, h0 + kh : h0 + kh + RPT, kw + 1 : kw + 1 + W]
                    nc.tensor.matmul(
                        acc[t][:P], w8[:, pos, :, ms : ms + P], rhs,
                        start=(cnt[t] == 0), stop=(cnt[t] == N_ACC - 1),
                        perf_mode=DR,
                    )
                    cnt[t] += 1

                def mm(lhsT, src, kh, kw, t, p0=0):
                    h0 = t * RPT
                    rhs = src[
                        p0 : p0 + lhsT.shape[0],
                        h0 + kh : h0 + kh + RPT,
                        kw + 1 : kw + 1 + W,
                    ]
                    nc.tensor.matmul(
                        acc[t][:P], lhsT, rhs,
                        start=(cnt[t] == 0), stop=(cnt[t] == N_ACC - 1),
                    )
                    cnt[t] += 1

                def pairs4(t):
                    for j, (pa, pb) in enumerate(TAIL_PAIRS):
                        src = xt[3] if (pa, pb) == (2, 3) else xt[2]
                        mm(wpair[:, j, ms : ms + P], src, pa // 3, pa % 3, t)

                def solo(t):
                    # row-tiled: even tiles on PE rows 0:64 (xp2 upper half),
                    # odd tiles on rows 64:128 (unshifted tail copy in xt4)
                    if t % 2 == 0:
                        mm(wpair[0:64, 4, ms : ms + P], xt[2], 2, 2, t)
                    else:
                        mm(wpair[64:P, 4, ms : ms + P], xt[4], 2, 2, t, p0=64)

                # DR phase first (all tiles), then the fp16 phase: the PE
                # pays ~200ns per fp16<->DR mode switch, so batch each mode.
                # The staged ramp micro-order matches chunk0's DMA arrival;
                # for chunk1 all data is resident and the order is harmless.
                for pos in range(3):
                    mm8(pos, 0)
                for t in range(1, 4):
                    for pos in range(3):
                        mm8(pos, t)
                for pos in range(3, P8):
                    for t in range(4):
                        mm8(pos, t)
                for pos in range(P8):
                    for t in range(4, NPT):
                        mm8(pos, t)
                for pos in range(P8, 9):
                    for t in range(NPT):
                        mm(wl0[:, pos - P8, ms : ms + P], xt[0],
                           pos // 3, pos % 3, t)
                for pos in range(P8, 9):
                    for t in range(NPT):
                        mm(wl1[:, pos - P8, ms : ms + P], xt[1],
                           pos // 3, pos % 3, t)
                for k in range(NPT // 2):
                    tA, tB = 2 * k, 2 * k + 1
                    pairs4(tA)
                    pairs4(tB)
                    solo(tA)
                    solo(tB)
                    for t in (tA, tB):
                        epi(acc[t][:P], mi,
                            [(out_d[ms : ms + P, t * NT : (t + 1) * NT], (0, P))])
                assert all(c == N_ACC for c in cnt), cnt

            full_chunk(0, 0, staged=True)
            full_chunk(P, 1, staged=True)

            # ---- co tail 256:320: column-tiled concurrent pixel-tile pairs,
            # all fp16 (DoubleRow and column tiling are mutually exclusive) ----
            cs = 256
            for k in range(4):
                tA, tB = 2 * k, 2 * k + 1
                pacc = pp.tile([P, NT], F32, tag=f"acc{k}", name=f"tacc{k}")
                cnt = [0]

                def pmm(lhsT, src, kh, kw, pacc=pacc, tA=tA, tB=tB, cnt=cnt):
                    first, last = cnt[0] == 0, cnt[0] == N_ACC2 - 1
                    np_ = lhsT.shape[0]
                    for col, t in ((0, tA), (64, tB)):
                        h0 = t * RPT
                        rhs = src[:np_, h0 + kh : h0 + kh + RPT, kw + 1 : kw + 1 + W]
                        nc.tensor.matmul(
                            pacc[col : col + 64], lhsT, rhs,
                            start=first, stop=last,
                            tile_position=(0, col),
                            skip_group_check=(col == 64),
                        )
                    cnt[0] += 1

                for pos in range(9):
                    lh = (w0t[:, pos, :] if pos < P8
                          else wl0[:, pos - P8, cs : cs + 64])
                    pmm(lh, xt[0], pos // 3, pos % 3)
                for pos in range(9):
                    lh = (w1t[:, pos, :] if pos < P8
                          else wl1[:, pos - P8, cs : cs + 64])
                    pmm(lh, xt[1], pos // 3, pos % 3)
                for j, (pa, pb) in enumerate(TAIL_PAIRS):
                    src = xt[3] if (pa, pb) == (2, 3) else xt[2]
                    pmm(wpair[:, j, cs : cs + 64], src, pa // 3, pa % 3)
                pmm(wpair[0:64, 4, cs : cs + 64], xt[2], 2, 2)
                assert cnt[0] == N_ACC2
                # last pair: drain its two output DMAs on idle queues so the
                # descriptor processing and transfers run in parallel (no MMs
                # remain, so no SBUF-port contention concern)
                epi(pacc[:P], 2,
                    [
                        (out_d[cs : cs + 64, tA * NT : (tA + 1) * NT], (0, 64)),
                        (out_d[cs : cs + 64, tB * NT : (tB + 1) * NT], (64, P)),
                    ],
                    queues=[nc.scalar, nc.sync] if k == 3 else None)

    nc.compile()
    return nc


_NC_CACHE = None


def _get_nc():
    global _NC_CACHE
    if _NC_CACHE is None:
        _NC_CACHE = build()
    return _NC_CACHE


def _prep_in_maps(x, w_bits, b_bits):
    # host-side hf8 decode (exact LUTs) + relayout [co,ci,kh,kw]->[ci,pos,co]
    codes = w_bits.astype(np.uint8)
    w9_16 = _LUT16[codes].transpose(1, 2, 3, 0).reshape(CIN, 9, COUT)  # *2048
    w9_8 = _LUT8[codes].transpose(1, 2, 3, 0).reshape(CIN, 9, COUT)  # *64 fp8
    # fp8 DR weights [128, P8, 2, COUT]
    w8v = np.ascontiguousarray(
        w9_8[0 : 2 * P, 0:P8]  # [256, P8, COUT]
        .reshape(2, P, P8, COUT)
        .transpose(1, 2, 0, 3)
    )
    # fp16 weights for pos P8..8
    w0 = np.ascontiguousarray(w9_16[0:P, P8:9])
    w1 = np.ascontiguousarray(w9_16[P : 2 * P, P8:9])
    # fp16 weights for pos 0..P8-1, couts 256:320 (col-tiled tail chunk)
    w0tv = np.ascontiguousarray(w9_16[0:P, 0:P8, 256:COUT])
    w1tv = np.ascontiguousarray(w9_16[P : 2 * P, 0:P8, 256:COUT])
    tail = w9_16[2 * P : CIN]  # [64, 9, 320]
    w2 = np.zeros((P, 5, COUT), np.float16)
    for j, (pa, pb) in enumerate(TAIL_PAIRS):
        w2[0:64, j] = tail[:, pa]
        w2[64:P, j] = tail[:, pb]
    w2[0:64, 4] = tail[:, 8]
    w2[64:P, 4] = tail[:, 8]
    b = _LUT32[b_bits.astype(np.uint8).reshape(COUT)]
    bfv = np.zeros((P, 4), np.float32)
    bfv[:, 0] = b[0:P]
    bfv[:, 1] = b[P : 2 * P]
    bfv[:, 2] = b[2 * P + (np.arange(P) % 64)]

    ins = []
    for i in range(B):
        xi = x[i].astype(np.float16)  # [320, 64, 64]
        xp = np.zeros((CIN, HP, WP), np.float16)
        xp[:, 1 : H + 1, 2 : W + 2] = xi
        # fp8 stacked image for ch 0:256 (scale 32)
        xq = np.clip(x[i, 0 : 2 * P].astype(np.float64) * 32.0, -240.0, 240.0)
        xq = xq.astype(ml_dtypes.float8_e4m3)
        x8v = np.zeros((P, 2, HP, WP8), ml_dtypes.float8_e4m3)
        x8v[:, 0, 1 : H + 1, 2 : W + 2] = xq[0:P]
        x8v[:, 1, 1 : H + 1, 2 : W + 2] = xq[P : 2 * P]
        xtail = xi[2 * P : CIN]  # [64, 64, 64]
        xp2 = np.zeros((P, HP, WP), np.float16)
        xp2[0:64] = xp[2 * P : CIN]
        xp2[64:P, 1 : H + 1, 1 : W + 1] = xtail  # shifted +1 col
        xb2 = np.zeros((P, HP, WP), np.float16)
        xb2[0:64] = xp[2 * P : CIN]
        xb2[64:P, 0:H, 4:WP] = xtail  # shifted +1 row, -2 col (flat +66)
        xc2 = np.ascontiguousarray(xp[2 * P : CIN])  # unshifted, for odd solos
        ins.append(
            {
                "x8": x8v,
                "xp0": np.ascontiguousarray(xp[0:P]),
                "xp1": np.ascontiguousarray(xp[P : 2 * P]),
                "xp2": xp2,
                "xb2": xb2,
                "xc2": xc2,
                "w8": w8v,
                "w0": w0,
                "w1": w1,
                "w0t": w0tv,
                "w1t": w1tv,
                "w2": w2,
                "bf": bfv,
            }
        )
    return ins


def kernel(x, w_bits, b_bits):
    nc = _get_nc()
    in_maps = _prep_in_maps(x, w_bits, b_bits)
    res = run_bass_kernel_spmd(nc, in_maps, core_ids=list(range(B)), trace=False)
    return np.stack(
        [res.results[i]["out"].reshape(COUT, H, W) for i in range(B)]
    ).astype(np.float32)


if __name__ == "__main__":
    rng = np.random.default_rng(0)
    x = rng.standard_normal((B, CIN, H, W)).astype(np.float32)
    w_bits = rng.integers(0, 256, (COUT, CIN, 3, 3)).astype(np.int32)
    b_bits = rng.integers(0, 256, (COUT,)).astype(np.int32)
    out = kernel(x, w_bits, b_bits)
    print("out", out.shape, out.dtype, float(np.abs(out).mean()))


# revision 20
# speedup vs baseline: 1.0347x; 1.0347x over previous
"""Trainium2 Bass kernel for nn_Conv2d_14147622273082.

Conv2d 3x3, stride 1, pad 1: x [8, 320, 64, 64] f32, hf8-coded weights
w_bits [320, 320, 3, 3] i32 (codes 0..255), bias codes b_bits [320] i32.
out = conv2d(x, hf8_decode(w_bits)) + hf8_decode(b_bits).

Strategy: data-parallel over batch (1 image per NeuronCore, 8 cores).
hf8 decode is a 256-entry LUT done host-side; weights are replicated.

Mixed fp8/fp16 matmul stream. The PE's DoubleRow fp8 mode contracts 256
rows per MM at the same issue rate as a 128-row fp16 MM (measured 222ns
either way at N=512), i.e. 2x throughput. Pure-fp8 x quantization costs
2.65% relative error (3 mantissa bits) vs the 2e-2 gate, so only P8=5 of
the 9 kernel positions (for channels 0:256) run in fp8 DoubleRow
(measured 1.77e-2 end-to-end); the rest stay fp16. Scales: weights are
hf8*64 in e4m3 (max exactly 240 = TRN e4m3 max normal, exact), x*32 in
e4m3, fp16 weights *2048 (power-of-2, exact) so both paths accumulate at
2048x in PSUM; the epilogue activation applies scale=1/2048 + bias and
writes fp16 (output downcast is ~2.4e-4, negligible vs 1.77e-2).

Per [128cout, 512pix] tile: 5 DR MMs (x8 stacked [128,2,66,72], blocks =
ch 0:128 / 128:256) + 8 fp16 MMs (pos 5..8 on xp0/xp1) + 4 tail-pair MMs
+ 1/2 row-tiled solo = 17.5 slots vs 22.5 all-fp16. Cout tail 256:320
is col-tiled fp16 pixel-tile pairs as before (DoubleRow and column
tiling are mutually exclusive on the XBUS budget).
"""

import numpy as np
import ml_dtypes

import concourse.bass as bass
import concourse.tile as tile
from concourse import bacc, mybir
from concourse.bass_utils import run_bass_kernel_spmd

B, CIN, COUT, H, W = 8, 320, 320, 64, 64
PIX = H * W  # 4096
P = 128
HP, WP = H + 2, W + 4  # 66 x 68 fp16 padded image
WP8 = 72  # fp8 stacked image width: 66*72 bytes per plane, %16 == 0
NT = 512  # pixels per psum tile = 8 rows of 64
RPT = NT // W  # 8
NPT = PIX // NT  # 8
P8 = 7  # kernel positions 0..P8-1 of ch 0:256 go fp8 DoubleRow
# tail position pairing: pos = kh*3+kw; pairs (a, b) packed on partitions
# (0:64, 64:128). Pairs with flat-offset delta +1 use xp2 (lower half
# pre-shifted +1 col); the (2,3) pair has delta +66 and uses xb2.
TAIL_PAIRS = [(0, 1), (2, 3), (4, 5), (6, 7)]
N_ACC = P8 + 2 * (9 - P8) + len(TAIL_PAIRS) + 1  # 18
N_ACC2 = 2 * 9 + len(TAIL_PAIRS) + 1  # 23 (col-tiled cout-tail chunk)

F16 = mybir.dt.float16
F32 = mybir.dt.float32
F8 = mybir.dt.float8e4
DR = mybir.MatmulPerfMode.DoubleRow
N_WARM = 72  # covers preamble->first-DMA latency and the HAM warm window
# (~3.4us of PE busy); the ramp DMA supply also gets a head start
WSCALE = 2048.0  # common PSUM scale: fp8 path 64*32, fp16 weights *2048


def _hf8_lut():
    bits = np.arange(256, dtype=np.int64)
    sign = np.where(((bits >> 7) & 1) == 1, -1.0, 1.0)
    exp = (bits >> 3) & 0xF
    man = (bits & 0x7).astype(np.float64)
    val = sign * np.where(
        exp == 0, 2.0 ** (1 - 14) * (man / 8.0), np.exp2(exp - 14.0) * (1 + man / 8.0)
    )
    return val


_LUT8 = (_hf8_lut() * 64.0).astype(ml_dtypes.float8_e4m3)  # max exactly 240
_LUT16 = (_hf8_lut() * WSCALE).astype(np.float16)  # exact (pow2 scale)
_LUT32 = _hf8_lut().astype(np.float32)


def build():
    from concourse.tile_rust import add_dep_helper

    nc = bacc.Bacc(
        "TRN2", target_bir_lowering=False, debug=False, enable_partition_id=False
    )
    x8_d = nc.dram_tensor("x8", [P, 2, HP, WP8], F8, kind="ExternalInput")
    xp_d = [
        nc.dram_tensor(f"xp{i}", [P, HP, WP], F16, kind="ExternalInput")
        for i in range(3)
    ]
    xb_d = nc.dram_tensor("xb2", [P, HP, WP], F16, kind="ExternalInput")
    xc_d = nc.dram_tensor("xc2", [64, HP, WP], F16, kind="ExternalInput")
    w8_d = nc.dram_tensor("w8", [P, P8, 2, COUT], F8, kind="ExternalInput")
    w0_d = nc.dram_tensor("w0", [P, 9 - P8, COUT], F16, kind="ExternalInput")
    w1_d = nc.dram_tensor("w1", [P, 9 - P8, COUT], F16, kind="ExternalInput")
    w0t_d = nc.dram_tensor("w0t", [P, P8, 64], F16, kind="ExternalInput")
    w1t_d = nc.dram_tensor("w1t", [P, P8, 64], F16, kind="ExternalInput")
    w2_d = nc.dram_tensor("w2", [P, 5, COUT], F16, kind="ExternalInput")
    bf_d = nc.dram_tensor("bf", [P, 4], F32, kind="ExternalInput")
    out_d = nc.dram_tensor("out", [COUT, PIX], F16, kind="ExternalOutput")

    with tile.TileContext(nc) as tc:
        with (
            tc.tile_pool(name="persist", bufs=1) as persist,
            tc.tile_pool(name="stage", bufs=1) as stage,
            tc.tile_pool(name="outsb", bufs=4) as outsb,
            tc.tile_pool(name="psum", bufs=1, space="PSUM") as pp,
        ):
            x8t = persist.tile([P, 2, HP, WP8], F8, tag="x8t", name="x8t")
            xt = [
                persist.tile([P, HP, WP], F16, tag=f"xt{i}", name=f"xt{i}")
                for i in range(5)
            ]
            w8 = persist.tile([P, P8, 2, COUT], F8, tag="w8", name="w8")
            wl0 = persist.tile([P, 9 - P8, COUT], F16, tag="wl0", name="wl0")
            wl1 = persist.tile([P, 9 - P8, COUT], F16, tag="wl1", name="wl1")
            w0t = persist.tile([P, P8, 64], F16, tag="w0t", name="w0t")
            w1t = persist.tile([P, P8, 64], F16, tag="w1t", name="w1t")
            wpair = persist.tile([P, 5, COUT], F16, tag="wpair", name="wpair")
            bf = persist.tile([P, 4], F32, tag="bf", name="bf")

            # ---- engine warmups (no data deps) ----
            wsrc = stage.tile([P, P], F16, tag="wsrc", name="wsrc")
            zsrc = stage.tile([P, 1], F32, tag="zsrc", name="zsrc")
            zo = stage.tile([P, 1], F32, tag="zo", name="zo")
            m0 = nc.gpsimd.memset(wsrc[:], 0.0)
            m1 = nc.gpsimd.memset(zsrc[:], 0.0)
            add_dep_helper(m1.ins, m0.ins, sync=False, reason="gpsimd order")
            act_warm = nc.scalar.activation(
                zo[:], zsrc[:], mybir.ActivationFunctionType.Identity, scale=1.0
            )

            # ---- input DMAs, deadline order, one in-order queue (a second
            # HWDGE queue for inputs measured strictly worse: the scalar
            # queue starts descriptor processing later than sync's) ----
            nc.sync.dma_start(x8t[:, :, 0:10], x8_d[:, :, 0:10])
            nc.sync.dma_start(w8[:, 0:1], w8_d[:, 0:1])
            nc.sync.dma_start(w8[:, 1:3], w8_d[:, 1:3])
            nc.sync.dma_start(x8t[:, :, 10:22], x8_d[:, :, 10:22])
            nc.sync.dma_start(x8t[:, :, 22:34], x8_d[:, :, 22:34])
            nc.sync.dma_start(w8[:, 3:5], w8_d[:, 3:5])
            nc.sync.dma_start(w8[:, 5:P8], w8_d[:, 5:P8])
            nc.sync.dma_start(x8t[:, :, 34:50], x8_d[:, :, 34:50])
            nc.sync.dma_start(x8t[:, :, 50:66], x8_d[:, :, 50:66])
            nc.sync.dma_start(xt[0][:, 0:16], xp_d[0][:, 0:16])
            nc.sync.dma_start(wl0[:], w0_d[:])
            nc.sync.dma_start(xt[0][:, 16:40], xp_d[0][:, 16:40])
            nc.sync.dma_start(xt[0][:, 40:66], xp_d[0][:, 40:66])
            nc.sync.dma_start(wl1[:], w1_d[:])
            nc.sync.dma_start(xt[1][:], xp_d[1][:])
            nc.sync.dma_start(wpair[:], w2_d[:])
            nc.sync.dma_start(xt[2][:], xp_d[2][:])
            nc.sync.dma_start(xt[3][:], xb_d[:])
            nc.sync.dma_start(xt[4][64:P], xc_d[:])
            nc.sync.dma_start(w0t[:], w0t_d[:])
            nc.sync.dma_start(w1t[:], w1t_d[:])
            nc.sync.dma_start(bf[:], bf_d[:])
            warm_ps = pp.tile([P, NT], F32, tag="acc7", name="warm_ps")
            for _ in range(N_WARM):
                nc.tensor.matmul(
                    warm_ps[0:64, 0:64], wsrc[:, 0:64], wsrc[:, 0:64],
                    start=True, stop=True,
                )

            # ---- matmul stream ----
            prev_act = {"a": act_warm}

            def epi(acc_t, bias_col, dsts, queues=None):
                osb = outsb.tile([P, NT], F16, tag="osb", name="osb")
                a = nc.scalar.activation(
                    osb[:], acc_t,
                    mybir.ActivationFunctionType.Identity,
                    bias=bf[:, bias_col : bias_col + 1], scale=1.0 / WSCALE,
                )
                add_dep_helper(
                    a.ins, prev_act["a"].ins, sync=False, reason="epi order"
                )
                prev_act["a"] = a
                for qi, (dst, rows) in enumerate(dsts):
                    q = queues[qi] if queues else nc.sync
                    q.dma_start(dst, osb[rows[0] : rows[1]])

            def full_chunk(ms, mi, staged):
                acc = [
                    pp.tile([P, NT], F32, tag=f"acc{t}", name=f"acc_{mi}_{t}")
                    for t in range(NPT)
                ]
                cnt = [0] * NPT

                def mm8(pos, t):
                    # DoubleRow fp8: contracts ch 0:256 for one position
                    kh, kw = pos // 3, pos % 3
                    h0 = t * RPT
                    rhs = x8t[:, :, h0 + kh : h0 + kh + RPT, kw + 1 : kw + 1 + W]
                    nc.tensor.matmul(
                        acc[t][:P], w8[:, pos, :, ms : ms + P], rhs,
                        start=(cnt[t] == 0), stop=(cnt[t] == N_ACC - 1),
                        perf_mode=DR,
                    )
                    cnt[t] += 1

                def mm(lhsT, src, kh, kw, t, p0=0):
                    h0 = t * RPT
                    rhs = src[
                        p0 : p0 + lhsT.shape[0],
                        h0 + kh : h0 + kh + RPT,
                        kw + 1 : kw + 1 + W,
                    ]
                    nc.tensor.matmul(
                        acc[t][:P], lhsT, rhs,
                        start=(cnt[t] == 0), stop=(cnt[t] == N_ACC - 1),
                    )
                    cnt[t] += 1

                def pairs4(t):
                    for j, (pa, pb) in enumerate(TAIL_PAIRS):
                        src = xt[3] if (pa, pb) == (2, 3) else xt[2]
                        mm(wpair[:, j, ms : ms + P], src, pa // 3, pa % 3, t)

                def solo(t):
                    # row-tiled: even tiles on PE rows 0:64 (xp2 upper half),
                    # odd tiles on rows 64:128 (unshifted tail copy in xt4)
                    if t % 2 == 0:
                        mm(wpair[0:64, 4, ms : ms + P], xt[2], 2, 2, t)
                    else:
                        mm(wpair[64:P, 4, ms : ms + P], xt[4], 2, 2, t, p0=64)

                # DR phase first (all tiles), then the fp16 phase: the PE
                # pays ~200ns per fp16<->DR mode switch, so batch each mode.
                # The staged ramp micro-order matches chunk0's DMA arrival;
                # for chunk1 all data is resident and the order is harmless.
                for pos in range(3):
                    mm8(pos, 0)
                for t in range(1, 4):
                    for pos in range(3):
                        mm8(pos, t)
                for pos in range(3, P8):
                    for t in range(4):
                        mm8(pos, t)
                for pos in range(P8):
                    for t in range(4, NPT):
                        mm8(pos, t)
                for pos in range(P8, 9):
                    for t in range(NPT):
                        mm(wl0[:, pos - P8, ms : ms + P], xt[0],
                           pos // 3, pos % 3, t)
                for pos in range(P8, 9):
                    for t in range(NPT):
                        mm(wl1[:, pos - P8, ms : ms + P], xt[1],
                           pos // 3, pos % 3, t)
                for k in range(NPT // 2):
                    tA, tB = 2 * k, 2 * k + 1
                    pairs4(tA)
                    pairs4(tB)
                    solo(tA)
                    solo(tB)
                    for t in (tA, tB):
                        epi(acc[t][:P], mi,
                            [(out_d[ms : ms + P, t * NT : (t + 1) * NT], (0, P))])
                assert all(c == N_ACC for c in cnt), cnt

            full_chunk(0, 0, staged=True)
            full_chunk(P, 1, staged=True)

            # ---- co tail 256:320: column-tiled concurrent pixel-tile pairs,
            # all fp16 (DoubleRow and column tiling are mutually exclusive) ----
            cs = 256
            for k in range(4):
                tA, tB = 2 * k, 2 * k + 1
                pacc = pp.tile([P, NT], F32, tag=f"acc{k}", name=f"tacc{k}")
                cnt = [0]

                def pmm(lhsT, src, kh, kw, pacc=pacc, tA=tA, tB=tB, cnt=cnt):
                    first, last = cnt[0] == 0, cnt[0] == N_ACC2 - 1
                    np_ = lhsT.shape[0]
                    for col, t in ((0, tA), (64, tB)):
                        h0 = t * RPT
                        rhs = src[:np_, h0 + kh : h0 + kh + RPT, kw + 1 : kw + 1 + W]
                        nc.tensor.matmul(
                            pacc[col : col + 64], lhsT, rhs,
                            start=first, stop=last,
                            tile_position=(0, col),
                            skip_group_check=(col == 64),
                        )
                    cnt[0] += 1

                for pos in range(9):
                    lh = (w0t[:, pos, :] if pos < P8
                          else wl0[:, pos - P8, cs : cs + 64])
                    pmm(lh, xt[0], pos // 3, pos % 3)
                for pos in range(9):
                    lh = (w1t[:, pos, :] if pos < P8
                          else wl1[:, pos - P8, cs : cs + 64])
                    pmm(lh, xt[1], pos // 3, pos % 3)
                for j, (pa, pb) in enumerate(TAIL_PAIRS):
                    src = xt[3] if (pa, pb) == (2, 3) else xt[2]
                    pmm(wpair[:, j, cs : cs + 64], src, pa // 3, pa % 3)
                pmm(wpair[0:64, 4, cs : cs + 64], xt[2], 2, 2)
                assert cnt[0] == N_ACC2
                # last pair: drain its two output DMAs on idle queues so the
                # descriptor processing and transfers run in parallel (no MMs
                # remain, so no SBUF-port contention concern)
                epi(pacc[:P], 2,
                    [
                        (out_d[cs : cs + 64, tA * NT : (tA + 1) * NT], (0, 64)),
                        (out_d[cs : cs + 64, tB * NT : (tB + 1) * NT], (64, P)),
                    ],
                    queues=[nc.scalar, nc.sync] if k == 3 else None)

    nc.compile()
    return nc


_NC_CACHE = None


def _get_nc():
    global _NC_CACHE
    if _NC_CACHE is None:
        _NC_CACHE = build()
    return _NC_CACHE


def _prep_in_maps(x, w_bits, b_bits):
    # host-side hf8 decode (exact LUTs) + relayout [co,ci,kh,kw]->[ci,pos,co]
    codes = w_bits.astype(np.uint8)
    w9_16 = _LUT16[codes].transpose(1, 2, 3, 0).reshape(CIN, 9, COUT)  # *2048
    w9_8 = _LUT8[codes].transpose(1, 2, 3, 0).reshape(CIN, 9, COUT)  # *64 fp8
    # fp8 DR weights [128, P8, 2, COUT]
    w8v = np.ascontiguousarray(
        w9_8[0 : 2 * P, 0:P8]  # [256, P8, COUT]
        .reshape(2, P, P8, COUT)
        .transpose(1, 2, 0, 3)
    )
    # fp16 weights for pos P8..8
    w0 = np.ascontiguousarray(w9_16[0:P, P8:9])
    w1 = np.ascontiguousarray(w9_16[P : 2 * P, P8:9])
    # fp16 weights for pos 0..P8-1, couts 256:320 (col-tiled tail chunk)
    w0tv = np.ascontiguousarray(w9_16[0:P, 0:P8, 256:COUT])
    w1tv = np.ascontiguousarray(w9_16[P : 2 * P, 0:P8, 256:COUT])
    tail = w9_16[2 * P : CIN]  # [64, 9, 320]
    w2 = np.zeros((P, 5, COUT), np.float16)
    for j, (pa, pb) in enumerate(TAIL_PAIRS):
        w2[0:64, j] = tail[:, pa]
        w2[64:P, j] = tail[:, pb]
    w2[0:64, 4] = tail[:, 8]
    w2[64:P, 4] = tail[:, 8]
    b = _LUT32[b_bits.astype(np.uint8).reshape(COUT)]
    bfv = np.zeros((P, 4), np.float32)
    bfv[:, 0] = b[0:P]
    bfv[:, 1] = b[P : 2 * P]
    bfv[:, 2] = b[2 * P + (np.arange(P) % 64)]

    ins = []
    for i in range(B):
        xi = x[i].astype(np.float16)  # [320, 64, 64]
        xp = np.zeros((CIN, HP, WP), np.float16)
        xp[:, 1 : H + 1, 2 : W + 2] = xi
        # fp8 stacked image for ch 0:256 (scale 32)
        xq = np.clip(x[i, 0 : 2 * P].astype(np.float64) * 32.0, -240.0, 240.0)
        xq = xq.astype(ml_dtypes.float8_e4m3)
        x8v = np.zeros((P, 2, HP, WP8), ml_dtypes.float8_e4m3)
        x8v[:, 0, 1 : H + 1, 2 : W + 2] = xq[0:P]
        x8v[:, 1, 1 : H + 1, 2 : W + 2] = xq[P : 2 * P]
        xtail = xi[2 * P : CIN]  # [64, 64, 64]
        xp2 = np.zeros((P, HP, WP), np.float16)
        xp2[0:64] = xp[2 * P : CIN]
        xp2[64:P, 1 : H + 1, 1 : W + 1] = xtail  # shifted +1 col
        xb2 = np.zeros((P, HP, WP), np.float16)
        xb2[0:64] = xp[2 * P : CIN]
        xb2[64:P, 0:H, 4:WP] = xtail  # shifted +1 row, -2 col (flat +66)
        xc2 = np.ascontiguousarray(xp[2 * P : CIN])  # unshifted, for odd solos
        ins.append(
            {
                "x8": x8v,
                "xp0": np.ascontiguousarray(xp[0:P]),
                "xp1": np.ascontiguousarray(xp[P : 2 * P]),
                "xp2": xp2,
                "xb2": xb2,
                "xc2": xc2,
                "w8": w8v,
                "w0": w0,
                "w1": w1,
                "w0t": w0tv,
                "w1t": w1tv,
                "w2": w2,
                "bf": bfv,
            }
        )
    return ins


def kernel(x, w_bits, b_bits):
    nc = _get_nc()
    in_maps = _prep_in_maps(x, w_bits, b_bits)
    res = run_bass_kernel_spmd(nc, in_maps, core_ids=list(range(B)), trace=False)
    return np.stack(
        [res.results[i]["out"].reshape(COUT, H, W) for i in range(B)]
    ).astype(np.float32)


if __name__ == "__main__":
    rng = np.random.default_rng(0)
    x = rng.standard_normal((B, CIN, H, W)).astype(np.float32)
    w_bits = rng.integers(0, 256, (COUT, CIN, 3, 3)).astype(np.int32)
    b_bits = rng.integers(0, 256, (COUT,)).astype(np.int32)
    out = kernel(x, w_bits, b_bits)
    print("out", out.shape, out.dtype, float(np.abs(out).mean()))
